# revision 23
# baseline (speedup 1.0000x reference)
"""MoE experts kernel for TRN2, expert-parallel over 8 NeuronCores.

Reference computation (T=4096, E=8, H=1024, Q=1024):
    gate_up = einsum('th,ehq->teq', x, gate_up_proj)      # (T, E, 2Q)
    gate, up = split(gate_up, 2, axis=-1)
    hidden = silu(gate) * up                              # (T, E, Q)
    expert_outputs = einsum('teq,eqh->teh', hidden, down_proj)
    out = einsum('teh,te->th', expert_outputs, routing_weights)

Sharding: expert-parallel. Core e computes its expert's full contribution
r[:, e] * (silu(x @ Wgu_gate) * (x @ Wgu_up)) @ Wdn  for all T tokens,
entirely in feature-major layout (features on partitions, tokens on the
free axis) so no on-device transposes are needed; the host sums the 8
partial outputs (the expert-parallel all-reduce) and transposes back.

Per-core cost model (measured):
  - 1536 bf16 matmuls of [128 contraction x 512 moving] at ~216ns each
    = 332us PE busy; 216ns is the effective clock floor (same-stationary
    matmuls are no faster). fp8 is no help twice over: its quantization
    error (3.8-6.5% end-to-end) eats the 2e-2 gate's margin, AND
    DoubleRow measures at most ~1.44x, so one residual-compensation
    matmul already makes it a net loss vs bf16.
  - Each dma_start costs ~2us fixed (completion receipt) + bytes at an
    effective rate set by the DRAM-side descriptor row size (~136GB/s at
    2KB rows), serialized per queue. So: few fat DMAs, with DRAM layouts
    host-packed so every transfer reads 4-16KB contiguous per partition.
    Three queues run concurrently: SP (nc.sync), Activation (nc.scalar),
    and SWDGE (nc.gpsimd).
  - The PE clock ramps over ~8 matmuls (~427ns each) after idling;
    dummy matmuls during the unavoidable initial DMA wait pay that cost
    off the critical path. A mid-stream starve >1us re-ramps.
  - ~7.5us fixed framework prologue before any DMA issues; ~1.8us
    epilogue barrier after the last store's semaphore.
"""

import sys

for _p in ("/opt/trn_rl_repo", "/root/.axon_site/_ro/trn_rl_repo"):
    if _p not in sys.path:
        sys.path.insert(0, _p)

import numpy as np

T, E, H, Q = 4096, 8, 1024, 1024
P = 128          # partitions
TC = 512         # token chunk (= one PSUM bank of fp32)
NT = T // TC     # 8 token chunks
KH = H // P      # 8 contraction tiles for the gate_up matmul
KQ = Q // P      # 8 contraction tiles for the down matmul
NH = H // P      # 8 output-feature tiles
NS = 2 * Q // P  # 16 gate_up weight slabs (gate qi / up qi interleaved)
N_WARM = 14      # PE clock-warmup dummies covering the first-DMA wait
BOOT = KH * TC // 2  # 2048: one boot third (4KB/partition) per DMA queue

_CACHED = None


def _split_waits(nc, max_waits=1):
    """Walrus codegen for several TRN2 ISA structs accepts only one sync-wait
    per instruction ("Too many sync wait commands"). Splitting is safe: a
    same-engine NoOp earlier in the (FIFO) stream carrying the extra waits
    blocks the stream at the same point the original multi-wait would have."""
    import concourse.mybir as mybir

    for f in nc.m.functions:
        for blk in f.blocks:
            newlist, changed = [], False
            for inst in blk.instructions:
                si = inst.sync_info
                if si is not None and si.on_wait and len(si.on_wait) > max_waits:
                    extra = si.on_wait[:-max_waits]
                    keep = si.on_wait[-max_waits:]
                    inst.sync_info = mybir.SyncInfo(
                        on_wait=list(keep), on_update=list(si.on_update or [])
                    )
                    for j, w in enumerate(extra):
                        nop = mybir.InstNoOp(
                            name=f"{inst.name}-wn{j}", engine=inst.engine
                        )
                        nop.sync_info = mybir.SyncInfo(on_wait=[w], on_update=[])
                        newlist.append(nop)
                    changed = True
                newlist.append(inst)
            if changed:
                blk.instructions = newlist


def _build():
    import concourse.bass as bass
    import concourse.mybir as mybir
    import concourse.tile as tile

    nc = bass.Bass("TRN2", target_bir_lowering=False, debug=False, num_devices=E)

    f32 = mybir.dt.float32
    # bf16: same PE rate as fp32r (1 cycle/row for moving >= 256) but half
    # the HBM traffic and half-width weight loads; quantization adds ~0.3%
    # relative error, well inside the 2e-2 gate.
    bf16 = mybir.dt.bfloat16

    # All DRAM layouts are host-packed so each DMA reads big contiguous
    # per-partition rows (see _make_in_maps):
    #   w_gu: quad-major [quad, P, slab-in-quad, KH, P]  -> 8KB rows/quad
    #   w_dn: [P, KQ, H]                                 -> 16KB rows
    #   xT:   [chunk, P, KH, TC]                         -> 8KB rows/chunk
    #   out:  [chunk, P, NH, TC]                         -> 8KB rows/chunk
    wq_d = nc.dram_tensor(
        "w_gu", [NS // 4, P, 4, KH, P], bf16, kind="ExternalInput"
    ).ap()
    wdn_d = nc.dram_tensor("w_dn", [P, KQ, H], bf16, kind="ExternalInput").ap()
    # chunks 1..7 of x; chunk 0 rides in the boot blob
    x_d = nc.dram_tensor(
        "xT", [NT - 1, P, KH, TC], bf16, kind="ExternalInput"
    ).ap()
    # boot blob: everything the first k-sweep needs, split into three equal
    # 4KB-per-partition thirds -- x0.k0-3 | slab0+slab1 | x0.k4-7 -- ONE fat
    # DMA per DMA queue (SP, ACT, SWDGE run concurrently at ~118GB/s each),
    # with the latest-deadline piece (x0.k4-7) on the slower SWDGE queue
    boot_d = nc.dram_tensor("boot", [3, P, BOOT], bf16, kind="ExternalInput").ap()
    rw_d = nc.dram_tensor("rw", [1, T], mybir.dt.float32, kind="ExternalInput").ap()
    # bf16 output: the host upcasts and sums the 8 expert partials in fp32;
    # the extra ~0.2% quantization is inside the 2e-2 budget and halves the
    # store traffic (shrinks the end-of-kernel DMA drain).
    out_d = nc.dram_tensor("out", [NT, P, NH, TC], bf16, kind="ExternalOutput").ap()

    from contextlib import ExitStack

    with tile.TileContext(nc) as tc:
        with ExitStack() as es:
            consts = es.enter_context(tc.tile_pool(name="consts", bufs=1))
            psum_gu = es.enter_context(tc.tile_pool(name="psum_gu", bufs=2, space="PSUM"))
            psum_o = es.enter_context(tc.tile_pool(name="psum_o", bufs=4, space="PSUM"))
            hid_pool = es.enter_context(tc.tile_pool(name="hid", bufs=2))
            tmp_pool = es.enter_context(tc.tile_pool(name="tmp", bufs=2))
            ost_pool = es.enter_context(tc.tile_pool(name="ost", bufs=2))
            wgu_s = consts.tile([P, NS, KH, P], bf16)
            wdn_s = consts.tile([P, KQ, H], bf16)
            # all x chunks stay SBUF-resident (62KB/partition with the boot
            # blob): no mid-stream x traffic at all after the startup loads
            x_s = consts.tile([P, NT - 1, KH, TC], bf16)
            boot_s = consts.tile([P, 3, BOOT], bf16)
            r_all = consts.tile([P, T], f32)

            def x0_mov(k):
                half, kk = divmod(k, 4)
                return boot_s[:, 2 * half, kk * TC:(kk + 1) * TC]

            def boot_slab(s, k):
                return boot_s[:, 1, s * KH * P + k * P:s * KH * P + (k + 1) * P]

            # PE p-state warmup: the engine idles from the end of the
            # framework prologue until the first weights+x land (~12.5us);
            # matmuls on a zeroed scratch tile during that window ramp the
            # clock so the real stream starts at full speed. Results land
            # in a PSUM bank that every real accumulation group resets
            # with start=True.
            dmy = consts.tile([P, 4, P], bf16)
            nc.gpsimd.memset(dmy, 0)
            warm_ps = psum_gu.tile([P, TC], f32, tag="gate")
            for _ in range(N_WARM):
                nc.tensor.matmul(
                    warm_ps, dmy[:, 0, :], dmy.rearrange("p a b -> p (a b)"),
                    start=True, stop=True,
                )

            # Startup schedule. The HBM aggregate (~320GB/s) is the binding
            # constraint and the queues round-robin for it, so: one boot-blob
            # DMA per HWDGE queue delivers the whole first k-sweep at once;
            # the rest of the weight stream (consumed at ~148GB/s by the PE)
            # is split 50/50 over the two fast queues as slab pairs in
            # consumption order; the slower SWDGE queue carries only traffic
            # whose deadline is far out, with x chunks 2-7 queued BEHIND
            # w_dn so they cannot steal bandwidth during the weight window.
            #   SP:    boot0 | pair45 | pair89   | pair12,13 | wdn.q0-3
            #          | stores 0,2,4,6
            #   ACT:   boot1 | pair67 | pair10,11 | pair14,15 | wdn.q4-7
            #          | routing bcast | stores 1,3,5
            #   SWDGE: boot2 | pair23 | x1 | x2 | x3-7
            nc.sync.dma_start(out=boot_s[:, 0], in_=boot_d[0])
            nc.scalar.dma_start(out=boot_s[:, 1], in_=boot_d[1])
            nc.gpsimd.dma_start(out=boot_s[:, 2], in_=boot_d[2])
            nc.sync.dma_start(out=wgu_s[:, 4:6], in_=wq_d[1, :, 0:2])
            nc.scalar.dma_start(out=wgu_s[:, 6:8], in_=wq_d[1, :, 2:4])
            nc.sync.dma_start(out=wgu_s[:, 8:10], in_=wq_d[2, :, 0:2])
            nc.scalar.dma_start(out=wgu_s[:, 10:12], in_=wq_d[2, :, 2:4])
            nc.sync.dma_start(out=wgu_s[:, 12:14], in_=wq_d[3, :, 0:2])
            nc.scalar.dma_start(out=wgu_s[:, 14:16], in_=wq_d[3, :, 2:4])
            nc.gpsimd.dma_start(out=wgu_s[:, 2:4], in_=wq_d[0, :, 2:4])
            nc.sync.dma_start(out=wdn_s[:, 0:KQ // 2], in_=wdn_d[:, 0:KQ // 2])
            nc.scalar.dma_start(out=wdn_s[:, KQ // 2:], in_=wdn_d[:, KQ // 2:])
            nc.gpsimd.dma_start(out=x_s[:, 0], in_=x_d[0])
            nc.gpsimd.dma_start(out=x_s[:, 1], in_=x_d[1])
            nc.gpsimd.dma_start(
                out=x_s[:, 2:7], in_=x_d[2:7].rearrange("c p k t -> p c k t")
            )
            nc.scalar.dma_start(out=r_all, in_=rw_d.to_broadcast([P, T]))
            nc.gpsimd.dma_start(
                out=r_all[:, 2 * TC:],
                in_=rw_d[:, 2 * TC:].to_broadcast([P, T - 2 * TC]),
            )

            for tci in range(NT):
                t0 = tci * TC

                def mov(k, tci=tci):
                    if tci == 0:
                        return x0_mov(k)
                    return x_s[:, tci - 1, k, :]

                def stat(s, k):
                    if s < 2:
                        # slabs 0/1 live in the boot blob, all chunks
                        return boot_slab(s, k)
                    return wgu_s[:, s, k, :]

                r_c = r_all[:, t0:t0 + TC]
                hid = hid_pool.tile([P, KQ, TC], bf16)
                for qi in range(KQ):
                    gate_ps = psum_gu.tile([P, TC], f32, tag="gate")
                    up_ps = psum_gu.tile([P, TC], f32, tag="up")
                    for k in range(KH):
                        nc.tensor.matmul(
                            gate_ps,
                            stat(2 * qi, k),
                            mov(k),
                            start=(k == 0),
                            stop=(k == KH - 1),
                        )
                    for k in range(KH):
                        nc.tensor.matmul(
                            up_ps,
                            stat(2 * qi + 1, k),
                            mov(k),
                            start=(k == 0),
                            stop=(k == KH - 1),
                        )
                    tmp = tmp_pool.tile([P, TC], f32)
                    nc.scalar.activation(
                        tmp, gate_ps, mybir.ActivationFunctionType.Silu
                    )
                    nc.vector.tensor_mul(hid[:, qi, :], tmp, up_ps)

                ost = ost_pool.tile([P, NH, TC], bf16, tag="ost")
                for hi in range(NH):
                    o_ps = psum_o.tile([P, TC], f32)
                    for qi in range(KQ):
                        nc.tensor.matmul(
                            o_ps,
                            wdn_s[:, qi, hi * P:(hi + 1) * P],
                            hid[:, qi, :],
                            start=(qi == 0),
                            stop=(qi == KQ - 1),
                        )
                    nc.vector.tensor_mul(ost[:, hi, :], o_ps, r_c)
                    if tci < NT - 1:
                        # one fat 1MB store per chunk (8KB rows), rings
                        # alternating so neither queue backs up
                        if hi == NH - 1:
                            eng = nc.sync if tci % 2 == 0 else nc.scalar
                            eng.dma_start(out=out_d[tci], in_=ost)
                    else:
                        # last chunk: split so the final piece after the last
                        # matmul is small and the two rings drain in parallel
                        if hi == 3:
                            nc.sync.dma_start(
                                out=out_d[tci, :, 0:4], in_=ost[:, 0:4]
                            )
                        elif hi == 6:
                            nc.scalar.dma_start(
                                out=out_d[tci, :, 4:7], in_=ost[:, 4:7]
                            )
                        elif hi == 7:
                            nc.sync.dma_start(
                                out=out_d[tci, :, 7:8], in_=ost[:, 7:8]
                            )
    _split_waits(nc)
    return nc


def _get_nc():
    global _CACHED
    if _CACHED is None:
        _CACHED = _build()
    return _CACHED


def _pack_wgu(w):
    """(H, 2Q) -> (4, P, 4, KH, P) bf16, quad-major in first-use slab order
    (gate qi / up qi interleaved), so a 4-slab quad reads 8KB contiguous per
    partition."""
    import ml_dtypes

    w = np.asarray(w, dtype=np.float32)
    # (KH, P, n_blk, P): k-tile, partition, column block, column
    w4 = w.reshape(KH, P, NS, P)
    order = [b for qi in range(KQ) for b in (qi, KQ + qi)]
    slabs = w4.transpose(2, 1, 0, 3)[order]          # (NS, P, KH, P)
    quads = slabs.reshape(NS // 4, 4, P, KH, P).transpose(0, 2, 1, 3, 4)
    return np.ascontiguousarray(quads.astype(ml_dtypes.bfloat16))


def _make_in_maps(x, routing_weights, gate_up_proj, down_proj):
    import ml_dtypes

    x = np.asarray(x, dtype=np.float32)
    # x[t, h] -> xP[chunk, p, k, t_in] with h = k*P + p: 8KB rows per chunk
    xP = x.reshape(NT, TC, KH, P).transpose(0, 3, 2, 1).astype(ml_dtypes.bfloat16)
    rw = np.asarray(routing_weights, dtype=np.float32)
    in_maps = []
    for e in range(E):
        dn = np.asarray(down_proj[e], dtype=np.float32)
        wq = _pack_wgu(gate_up_proj[e])
        # boot thirds: x0.k0-3 | slab0+slab1 | x0.k4-7, 4KB per partition
        boot = np.stack([
            xP[0, :, 0:4].reshape(P, BOOT),
            wq[0, :, 0:2].reshape(P, BOOT),
            xP[0, :, 4:8].reshape(P, BOOT),
        ])
        in_maps.append({
            "xT": np.ascontiguousarray(xP[1:]),
            "w_gu": wq,
            "boot": np.ascontiguousarray(boot),
            # w_dn[p, qi, h] = down_proj[qi*P + p, h]: 16KB rows
            "w_dn": np.ascontiguousarray(
                dn.reshape(KQ, P, H).transpose(1, 0, 2).astype(ml_dtypes.bfloat16)
            ),
            "rw": np.ascontiguousarray(rw[:, e].reshape(1, T)),
        })
    return in_maps


def _reduce_out(res):
    total = np.zeros((NT, P, NH, TC), dtype=np.float32)
    for r in res.results:
        total += r["out"].astype(np.float32).reshape(NT, P, NH, TC)
    # [chunk, p, hi, t_in] -> (T, H) with h = hi*P + p
    return np.ascontiguousarray(
        total.transpose(0, 3, 2, 1).reshape(T, H)
    )


def kernel(x, routing_weights, gate_up_proj, down_proj):
    from concourse.bass_utils import run_bass_kernel_spmd

    nc = _get_nc()
    in_maps = _make_in_maps(x, routing_weights, gate_up_proj, down_proj)
    res = run_bass_kernel_spmd(nc, in_maps, core_ids=list(range(E)))
    return _reduce_out(res)


# revision 29
# speedup vs baseline: 1.0055x; 1.0055x over previous
"""MoE experts kernel for TRN2, expert-parallel over 8 NeuronCores.

Reference computation (T=4096, E=8, H=1024, Q=1024):
    gate_up = einsum('th,ehq->teq', x, gate_up_proj)      # (T, E, 2Q)
    gate, up = split(gate_up, 2, axis=-1)
    hidden = silu(gate) * up                              # (T, E, Q)
    expert_outputs = einsum('teq,eqh->teh', hidden, down_proj)
    out = einsum('teh,te->th', expert_outputs, routing_weights)

Sharding: expert-parallel. Core e computes its expert's full contribution
r[:, e] * (silu(x @ Wgu_gate) * (x @ Wgu_up)) @ Wdn  for all T tokens,
entirely in feature-major layout (features on partitions, tokens on the
free axis) so no on-device transposes are needed; the host sums the 8
partial outputs (the expert-parallel all-reduce) and transposes back.

Per-core cost model (measured):
  - 1536 bf16 matmuls of [128 contraction x 512 moving] at ~216ns each
    = 332us PE busy; 216ns is the effective clock floor (same-stationary
    matmuls are no faster). fp8 is no help twice over: its quantization
    error (3.8-6.5% end-to-end) eats the 2e-2 gate's margin, AND
    DoubleRow measures at most ~1.44x, so one residual-compensation
    matmul already makes it a net loss vs bf16.
  - Each dma_start costs ~2us fixed (completion receipt) + bytes at an
    effective rate set by the DRAM-side descriptor row size (~136GB/s at
    2KB rows), serialized per queue. So: few fat DMAs, with DRAM layouts
    host-packed so every transfer reads 4-16KB contiguous per partition.
    Three queues run concurrently: SP (nc.sync), Activation (nc.scalar),
    and SWDGE (nc.gpsimd).
  - The PE clock ramps over ~8 matmuls (~427ns each) after idling;
    dummy matmuls during the unavoidable initial DMA wait pay that cost
    off the critical path. A mid-stream starve >1us re-ramps.
  - ~7.5us fixed framework prologue before any DMA issues; ~1.8us
    epilogue barrier after the last store's semaphore.
"""

import sys

for _p in ("/opt/trn_rl_repo", "/root/.axon_site/_ro/trn_rl_repo"):
    if _p not in sys.path:
        sys.path.insert(0, _p)

import numpy as np

T, E, H, Q = 4096, 8, 1024, 1024
P = 128          # partitions
TC = 512         # token chunk (= one PSUM bank of fp32)
NT = T // TC     # 8 token chunks
KH = H // P      # 8 contraction tiles for the gate_up matmul
KQ = Q // P      # 8 contraction tiles for the down matmul
NH = H // P      # 8 output-feature tiles
NS = 2 * Q // P  # 16 gate_up weight slabs (gate qi / up qi interleaved)
N_WARM = 18      # PE clock-warmup dummies covering the first-DMA wait
BOOT = KH * TC // 2 + KH * P  # 3072: half of x chunk 0 + one slab, per queue

_CACHED = None


def _split_waits(nc, max_waits=1):
    """Walrus codegen for several TRN2 ISA structs accepts only one sync-wait
    per instruction ("Too many sync wait commands"). Splitting is safe: a
    same-engine NoOp earlier in the (FIFO) stream carrying the extra waits
    blocks the stream at the same point the original multi-wait would have."""
    import concourse.mybir as mybir

    for f in nc.m.functions:
        for blk in f.blocks:
            newlist, changed = [], False
            for inst in blk.instructions:
                si = inst.sync_info
                if si is not None and si.on_wait and len(si.on_wait) > max_waits:
                    extra = si.on_wait[:-max_waits]
                    keep = si.on_wait[-max_waits:]
                    inst.sync_info = mybir.SyncInfo(
                        on_wait=list(keep), on_update=list(si.on_update or [])
                    )
                    for j, w in enumerate(extra):
                        nop = mybir.InstNoOp(
                            name=f"{inst.name}-wn{j}", engine=inst.engine
                        )
                        nop.sync_info = mybir.SyncInfo(on_wait=[w], on_update=[])
                        newlist.append(nop)
                    changed = True
                newlist.append(inst)
            if changed:
                blk.instructions = newlist


def _build():
    import concourse.bass as bass
    import concourse.mybir as mybir
    import concourse.tile as tile

    nc = bass.Bass("TRN2", target_bir_lowering=False, debug=False, num_devices=E)

    f32 = mybir.dt.float32
    # bf16: same PE rate as fp32r (1 cycle/row for moving >= 256) but half
    # the HBM traffic and half-width weight loads; quantization adds ~0.3%
    # relative error, well inside the 2e-2 gate.
    bf16 = mybir.dt.bfloat16

    # All DRAM layouts are host-packed so each DMA reads big contiguous
    # per-partition rows (see _make_in_maps):
    #   w_gu: quad-major [quad, P, slab-in-quad, KH, P]  -> 8KB rows/quad
    #   w_dn: [P, KQ, H]                                 -> 16KB rows
    #   xT:   [chunk, P, KH, TC]                         -> 8KB rows/chunk
    #   out:  [chunk, P, NH, TC]                         -> 8KB rows/chunk
    wq_d = nc.dram_tensor(
        "w_gu", [NS // 4, P, 4, KH, P], bf16, kind="ExternalInput"
    ).ap()
    wdn_d = nc.dram_tensor("w_dn", [P, KQ, H], bf16, kind="ExternalInput").ap()
    # chunks 1..7 of x; chunk 0 rides in the boot blob
    x_d = nc.dram_tensor(
        "xT", [NT - 1, P, KH, TC], bf16, kind="ExternalInput"
    ).ap()
    # boot blob: everything the first k-sweep needs, packed per partition as
    # [x0.k0-3 | slab0] and [x0.k4-7 | slab1] so ONE fat DMA per HWDGE queue
    # (6KB rows) pays the ~2us fixed cost once and lands it all together
    boot_d = nc.dram_tensor("boot", [2, P, BOOT], bf16, kind="ExternalInput").ap()
    rw_d = nc.dram_tensor("rw", [1, T], mybir.dt.float32, kind="ExternalInput").ap()
    # bf16 output: the host upcasts and sums the 8 expert partials in fp32;
    # the extra ~0.2% quantization is inside the 2e-2 budget and halves the
    # store traffic (shrinks the end-of-kernel DMA drain).
    out_d = nc.dram_tensor("out", [NT, P, NH, TC], bf16, kind="ExternalOutput").ap()

    from contextlib import ExitStack

    with tile.TileContext(nc) as tc:
        with ExitStack() as es:
            consts = es.enter_context(tc.tile_pool(name="consts", bufs=1))
            psum_gu = es.enter_context(tc.tile_pool(name="psum_gu", bufs=2, space="PSUM"))
            psum_o = es.enter_context(tc.tile_pool(name="psum_o", bufs=4, space="PSUM"))
            hid_pool = es.enter_context(tc.tile_pool(name="hid", bufs=2))
            tmp_pool = es.enter_context(tc.tile_pool(name="tmp", bufs=2))
            ost_pool = es.enter_context(tc.tile_pool(name="ost", bufs=2))
            wgu_s = consts.tile([P, NS, KH, P], bf16)
            wdn_s = consts.tile([P, KQ, H], bf16)
            # all x chunks stay SBUF-resident (62KB/partition with the boot
            # blob): no mid-stream x traffic at all after the startup loads
            x_s = consts.tile([P, NT - 1, KH, TC], bf16)
            boot_s = consts.tile([P, 2, BOOT], bf16)
            r_all = consts.tile([P, T], f32)

            def x0_mov(k):
                half, kk = divmod(k, 4)
                return boot_s[:, half, kk * TC:(kk + 1) * TC]

            def boot_slab(s, k):
                return boot_s[:, s, 4 * TC + k * P:4 * TC + (k + 1) * P]

            # PE p-state warmup: the engine idles from the end of the
            # framework prologue until the first weights+x land (~12.5us);
            # matmuls on a zeroed scratch tile during that window ramp the
            # clock so the real stream starts at full speed. Results land
            # in a PSUM bank that every real accumulation group resets
            # with start=True.
            dmy = consts.tile([P, 4, P], bf16)
            nc.gpsimd.memset(dmy, 0)
            warm_ps = psum_gu.tile([P, TC], f32, tag="gate")
            for _ in range(N_WARM):
                nc.tensor.matmul(
                    warm_ps, dmy[:, 0, :], dmy.rearrange("p a b -> p (a b)"),
                    start=True, stop=True,
                )

            # Startup schedule. The HBM aggregate (~320GB/s) is the binding
            # constraint and the queues round-robin for it, so: one boot-blob
            # DMA per HWDGE queue delivers the whole first k-sweep at once;
            # the rest of the weight stream (consumed at ~148GB/s by the PE)
            # is split 50/50 over the two fast queues as slab pairs in
            # consumption order; the slower SWDGE queue carries only traffic
            # whose deadline is far out, with x chunks 2-7 queued BEHIND
            # w_dn so they cannot steal bandwidth during the weight window.
            #   SP:    boot0 | pair45 | pair89   | pair12,13 | stores 0,2,4,6
            #   ACT:   boot1 | pair67 | pair10,11 | pair14,15
            #          | routing bcast | stores 1,3,5
            #   SWDGE: pair23 | w_dn | x1 | x2 | ... | x7
            nc.sync.dma_start(out=boot_s[:, 0], in_=boot_d[0])
            nc.scalar.dma_start(out=boot_s[:, 1], in_=boot_d[1])
            nc.gpsimd.dma_start(out=wgu_s[:, 2:4], in_=wq_d[0, :, 2:4])
            nc.sync.dma_start(out=wgu_s[:, 4:6], in_=wq_d[1, :, 0:2])
            nc.scalar.dma_start(out=wgu_s[:, 6:8], in_=wq_d[1, :, 2:4])
            nc.sync.dma_start(out=wgu_s[:, 8:10], in_=wq_d[2, :, 0:2])
            nc.scalar.dma_start(out=wgu_s[:, 10:12], in_=wq_d[2, :, 2:4])
            nc.sync.dma_start(out=wgu_s[:, 12:14], in_=wq_d[3, :, 0:2])
            nc.scalar.dma_start(out=wgu_s[:, 14:16], in_=wq_d[3, :, 2:4])
            nc.gpsimd.dma_start(out=wdn_s, in_=wdn_d)
            for c in range(1, NT):
                nc.gpsimd.dma_start(out=x_s[:, c - 1], in_=x_d[c - 1])
            nc.scalar.dma_start(out=r_all, in_=rw_d.to_broadcast([P, T]))
            nc.gpsimd.dma_start(
                out=r_all[:, 2 * TC:],
                in_=rw_d[:, 2 * TC:].to_broadcast([P, T - 2 * TC]),
            )

            for tci in range(NT):
                t0 = tci * TC

                def mov(k, tci=tci):
                    if tci == 0:
                        return x0_mov(k)
                    return x_s[:, tci - 1, k, :]

                def stat(s, k):
                    if s < 2:
                        # slabs 0/1 live in the boot blob, all chunks
                        return boot_slab(s, k)
                    return wgu_s[:, s, k, :]

                r_c = r_all[:, t0:t0 + TC]
                hid = hid_pool.tile([P, KQ, TC], bf16)
                for qi in range(KQ):
                    gate_ps = psum_gu.tile([P, TC], f32, tag="gate")
                    up_ps = psum_gu.tile([P, TC], f32, tag="up")
                    for k in range(KH):
                        nc.tensor.matmul(
                            gate_ps,
                            stat(2 * qi, k),
                            mov(k),
                            start=(k == 0),
                            stop=(k == KH - 1),
                        )
                    for k in range(KH):
                        nc.tensor.matmul(
                            up_ps,
                            stat(2 * qi + 1, k),
                            mov(k),
                            start=(k == 0),
                            stop=(k == KH - 1),
                        )
                    tmp = tmp_pool.tile([P, TC], f32)
                    nc.scalar.activation(
                        tmp, gate_ps, mybir.ActivationFunctionType.Silu
                    )
                    nc.vector.tensor_mul(hid[:, qi, :], tmp, up_ps)

                ost = ost_pool.tile([P, NH, TC], bf16, tag="ost")
                for hi in range(NH):
                    o_ps = psum_o.tile([P, TC], f32)
                    if tci == NT - 1 and hi == NH - 1:
                        # final tile in two 256-token halves: the first half
                        # streams out on the idle SWDGE queue while the PE
                        # finishes the second, so the end-of-kernel drain is
                        # one 64KB store instead of 128KB
                        for half in range(2):
                            hs = slice(half * (TC // 2), (half + 1) * (TC // 2))
                            for qi in range(KQ):
                                nc.tensor.matmul(
                                    o_ps[:, hs],
                                    wdn_s[:, qi, hi * P:(hi + 1) * P],
                                    hid[:, qi, hs],
                                    start=(qi == 0),
                                    stop=(qi == KQ - 1),
                                )
                            nc.vector.tensor_mul(
                                ost[:, hi, hs], o_ps[:, hs], r_c[:, hs]
                            )
                            eng = nc.gpsimd if half == 0 else nc.sync
                            eng.dma_start(
                                out=out_d[tci, :, hi:hi + 1, hs],
                                in_=ost[:, hi:hi + 1, hs],
                            )
                        continue
                    for qi in range(KQ):
                        nc.tensor.matmul(
                            o_ps,
                            wdn_s[:, qi, hi * P:(hi + 1) * P],
                            hid[:, qi, :],
                            start=(qi == 0),
                            stop=(qi == KQ - 1),
                        )
                    nc.vector.tensor_mul(ost[:, hi, :], o_ps, r_c)
                    if tci < NT - 1:
                        # one fat 1MB store per chunk (8KB rows), rings
                        # alternating so neither queue backs up
                        if hi == NH - 1:
                            eng = nc.sync if tci % 2 == 0 else nc.scalar
                            eng.dma_start(out=out_d[tci], in_=ost)
                    else:
                        # last chunk: split so the final piece after the last
                        # matmul is small and the two rings drain in parallel
                        if hi == 3:
                            nc.sync.dma_start(
                                out=out_d[tci, :, 0:4], in_=ost[:, 0:4]
                            )
                        elif hi == 6:
                            nc.scalar.dma_start(
                                out=out_d[tci, :, 4:7], in_=ost[:, 4:7]
                            )
    _split_waits(nc)
    return nc


def _get_nc():
    global _CACHED
    if _CACHED is None:
        _CACHED = _build()
    return _CACHED


def _pack_wgu(w):
    """(H, 2Q) -> (4, P, 4, KH, P) bf16, quad-major in first-use slab order
    (gate qi / up qi interleaved), so a 4-slab quad reads 8KB contiguous per
    partition."""
    import ml_dtypes

    w = np.asarray(w, dtype=np.float32)
    # (KH, P, n_blk, P): k-tile, partition, column block, column
    w4 = w.reshape(KH, P, NS, P)
    order = [b for qi in range(KQ) for b in (qi, KQ + qi)]
    slabs = w4.transpose(2, 1, 0, 3)[order]          # (NS, P, KH, P)
    quads = slabs.reshape(NS // 4, 4, P, KH, P).transpose(0, 2, 1, 3, 4)
    return np.ascontiguousarray(quads.astype(ml_dtypes.bfloat16))


def _make_in_maps(x, routing_weights, gate_up_proj, down_proj):
    import ml_dtypes

    x = np.asarray(x, dtype=np.float32)
    # x[t, h] -> xP[chunk, p, k, t_in] with h = k*P + p: 8KB rows per chunk
    xP = x.reshape(NT, TC, KH, P).transpose(0, 3, 2, 1).astype(ml_dtypes.bfloat16)
    rw = np.asarray(routing_weights, dtype=np.float32)
    in_maps = []
    for e in range(E):
        dn = np.asarray(down_proj[e], dtype=np.float32)
        wq = _pack_wgu(gate_up_proj[e])
        # boot half s: [x0.k(4s..4s+3) flat | slab s], 6KB per partition
        boot = np.concatenate(
            [
                xP[0].reshape(P, 2, KH // 2 * TC).transpose(1, 0, 2),
                wq[0, :, 0:2].reshape(P, 2, KH * P).transpose(1, 0, 2),
            ],
            axis=2,
        )
        in_maps.append({
            "xT": np.ascontiguousarray(xP[1:]),
            "w_gu": wq,
            "boot": np.ascontiguousarray(boot),
            # w_dn[p, qi, h] = down_proj[qi*P + p, h]: 16KB rows
            "w_dn": np.ascontiguousarray(
                dn.reshape(KQ, P, H).transpose(1, 0, 2).astype(ml_dtypes.bfloat16)
            ),
            "rw": np.ascontiguousarray(rw[:, e].reshape(1, T)),
        })
    return in_maps


def _reduce_out(res):
    total = np.zeros((NT, P, NH, TC), dtype=np.float32)
    for r in res.results:
        total += r["out"].astype(np.float32).reshape(NT, P, NH, TC)
    # [chunk, p, hi, t_in] -> (T, H) with h = hi*P + p
    return np.ascontiguousarray(
        total.transpose(0, 3, 2, 1).reshape(T, H)
    )


def kernel(x, routing_weights, gate_up_proj, down_proj):
    from concourse.bass_utils import run_bass_kernel_spmd

    nc = _get_nc()
    in_maps = _make_in_maps(x, routing_weights, gate_up_proj, down_proj)
    res = run_bass_kernel_spmd(nc, in_maps, core_ids=list(range(E)))
    return _reduce_out(res)


# revision 33
# speedup vs baseline: 1.0092x; 1.0037x over previous
"""MoE experts kernel for TRN2, expert-parallel over 8 NeuronCores.

Reference computation (T=4096, E=8, H=1024, Q=1024):
    gate_up = einsum('th,ehq->teq', x, gate_up_proj)      # (T, E, 2Q)
    gate, up = split(gate_up, 2, axis=-1)
    hidden = silu(gate) * up                              # (T, E, Q)
    expert_outputs = einsum('teq,eqh->teh', hidden, down_proj)
    out = einsum('teh,te->th', expert_outputs, routing_weights)

Sharding: expert-parallel. Core e computes its expert's full contribution
r[:, e] * (silu(x @ Wgu_gate) * (x @ Wgu_up)) @ Wdn  for all T tokens,
entirely in feature-major layout (features on partitions, tokens on the
free axis) so no on-device transposes are needed; the host sums the 8
partial outputs (the expert-parallel all-reduce) and transposes back.

Per-core cost model (measured):
  - 1536 bf16 matmuls of [128 contraction x 512 moving] at ~216ns each
    = 332us PE busy; 216ns is the effective clock floor (same-stationary
    matmuls are no faster). fp8 is no help twice over: its quantization
    error (3.8-6.5% end-to-end) eats the 2e-2 gate's margin, AND
    DoubleRow measures at most ~1.44x, so one residual-compensation
    matmul already makes it a net loss vs bf16.
  - Each dma_start costs ~2us fixed (completion receipt) + bytes at an
    effective rate set by the DRAM-side descriptor row size (~136GB/s at
    2KB rows), serialized per queue. So: few fat DMAs, with DRAM layouts
    host-packed so every transfer reads 4-16KB contiguous per partition.
    Three queues run concurrently: SP (nc.sync), Activation (nc.scalar),
    and SWDGE (nc.gpsimd).
  - The PE clock ramps over ~8 matmuls (~427ns each) after idling;
    dummy matmuls during the unavoidable initial DMA wait pay that cost
    off the critical path. A mid-stream starve >1us re-ramps.
  - ~7.5us fixed framework prologue before any DMA issues; ~1.8us
    epilogue barrier after the last store's semaphore.
"""

import sys

for _p in ("/opt/trn_rl_repo", "/root/.axon_site/_ro/trn_rl_repo"):
    if _p not in sys.path:
        sys.path.insert(0, _p)

import numpy as np

T, E, H, Q = 4096, 8, 1024, 1024
P = 128          # partitions
TC = 512         # token chunk (= one PSUM bank of fp32)
NT = T // TC     # 8 token chunks
KH = H // P      # 8 contraction tiles for the gate_up matmul
KQ = Q // P      # 8 contraction tiles for the down matmul
NH = H // P      # 8 output-feature tiles
NS = 2 * Q // P  # 16 gate_up weight slabs (gate qi / up qi interleaved)
N_WARM = 18      # PE clock-warmup dummies covering the first-DMA wait
BOOT = KH * TC // 2 + KH * P  # 3072: half of x chunk 0 + one slab, per queue

_CACHED = None


def _split_waits(nc, max_waits=1):
    """Walrus codegen for several TRN2 ISA structs accepts only one sync-wait
    per instruction ("Too many sync wait commands"). Splitting is safe: a
    same-engine NoOp earlier in the (FIFO) stream carrying the extra waits
    blocks the stream at the same point the original multi-wait would have."""
    import concourse.mybir as mybir

    for f in nc.m.functions:
        for blk in f.blocks:
            newlist, changed = [], False
            for inst in blk.instructions:
                si = inst.sync_info
                if si is not None and si.on_wait and len(si.on_wait) > max_waits:
                    extra = si.on_wait[:-max_waits]
                    keep = si.on_wait[-max_waits:]
                    inst.sync_info = mybir.SyncInfo(
                        on_wait=list(keep), on_update=list(si.on_update or [])
                    )
                    for j, w in enumerate(extra):
                        nop = mybir.InstNoOp(
                            name=f"{inst.name}-wn{j}", engine=inst.engine
                        )
                        nop.sync_info = mybir.SyncInfo(on_wait=[w], on_update=[])
                        newlist.append(nop)
                    changed = True
                newlist.append(inst)
            if changed:
                blk.instructions = newlist


def _build():
    import concourse.bass as bass
    import concourse.mybir as mybir
    import concourse.tile as tile

    nc = bass.Bass("TRN2", target_bir_lowering=False, debug=False, num_devices=E)

    f32 = mybir.dt.float32
    # bf16: same PE rate as fp32r (1 cycle/row for moving >= 256) but half
    # the HBM traffic and half-width weight loads; quantization adds ~0.3%
    # relative error, well inside the 2e-2 gate.
    bf16 = mybir.dt.bfloat16

    # All DRAM layouts are host-packed so each DMA reads big contiguous
    # per-partition rows (see _make_in_maps):
    #   w_gu: quad-major [quad, P, slab-in-quad, KH, P]  -> 8KB rows/quad
    #   w_dn: [P, KQ, H]                                 -> 16KB rows
    #   xT:   [chunk, P, KH, TC]                         -> 8KB rows/chunk
    #   out:  [chunk, P, NH, TC]                         -> 8KB rows/chunk
    wq_d = nc.dram_tensor(
        "w_gu", [NS // 4, P, 4, KH, P], bf16, kind="ExternalInput"
    ).ap()
    wdn_d = nc.dram_tensor("w_dn", [P, KQ, H], bf16, kind="ExternalInput").ap()
    # chunks 1..7 of x; chunk 0 rides in the boot blob
    x_d = nc.dram_tensor(
        "xT", [NT - 1, P, KH, TC], bf16, kind="ExternalInput"
    ).ap()
    # boot blob: everything the first k-sweep needs, packed per partition as
    # [x0.k0-3 | slab0] and [x0.k4-7 | slab1] so ONE fat DMA per HWDGE queue
    # (6KB rows) pays the ~2us fixed cost once and lands it all together
    boot_d = nc.dram_tensor("boot", [2, P, BOOT], bf16, kind="ExternalInput").ap()
    rw_d = nc.dram_tensor("rw", [1, T], mybir.dt.float32, kind="ExternalInput").ap()
    # bf16 output: the host upcasts and sums the 8 expert partials in fp32;
    # the extra ~0.2% quantization is inside the 2e-2 budget and halves the
    # store traffic (shrinks the end-of-kernel DMA drain).
    out_d = nc.dram_tensor("out", [NT, P, NH, TC], bf16, kind="ExternalOutput").ap()

    from contextlib import ExitStack

    with tile.TileContext(nc) as tc:
        with ExitStack() as es:
            consts = es.enter_context(tc.tile_pool(name="consts", bufs=1))
            psum_gu = es.enter_context(tc.tile_pool(name="psum_gu", bufs=2, space="PSUM"))
            psum_o = es.enter_context(tc.tile_pool(name="psum_o", bufs=4, space="PSUM"))
            hid_pool = es.enter_context(tc.tile_pool(name="hid", bufs=2))
            tmp_pool = es.enter_context(tc.tile_pool(name="tmp", bufs=2))
            ost_pool = es.enter_context(tc.tile_pool(name="ost", bufs=2))
            wgu_s = consts.tile([P, NS, KH, P], bf16)
            wdn_s = consts.tile([P, KQ, H], bf16)
            # all x chunks stay SBUF-resident (62KB/partition with the boot
            # blob): no mid-stream x traffic at all after the startup loads
            x_s = consts.tile([P, NT - 1, KH, TC], bf16)
            boot_s = consts.tile([P, 2, BOOT], bf16)
            r_all = consts.tile([P, T], f32)

            def x0_mov(k):
                half, kk = divmod(k, 4)
                return boot_s[:, half, kk * TC:(kk + 1) * TC]

            def boot_slab(s, k):
                return boot_s[:, s, 4 * TC + k * P:4 * TC + (k + 1) * P]

            # PE p-state warmup: the engine idles from the end of the
            # framework prologue until the first weights+x land (~12.5us);
            # matmuls on a zeroed scratch tile during that window ramp the
            # clock so the real stream starts at full speed. Results land
            # in a PSUM bank that every real accumulation group resets
            # with start=True.
            dmy = consts.tile([P, 4, P], bf16)
            nc.gpsimd.memset(dmy, 0)
            warm_ps = psum_gu.tile([P, TC], f32, tag="gate")
            for _ in range(N_WARM):
                nc.tensor.matmul(
                    warm_ps, dmy[:, 0, :], dmy.rearrange("p a b -> p (a b)"),
                    start=True, stop=True,
                )

            # Startup schedule. The HBM aggregate (~320GB/s) is the binding
            # constraint and the queues round-robin for it, so: one boot-blob
            # DMA per HWDGE queue delivers the whole first k-sweep at once;
            # the rest of the weight stream (consumed at ~148GB/s by the PE)
            # is split 50/50 over the two fast queues as slab pairs in
            # consumption order; the slower SWDGE queue carries only traffic
            # whose deadline is far out, with x chunks 2-7 queued BEHIND
            # w_dn so they cannot steal bandwidth during the weight window.
            #   SP:    boot0 | pair45 | pair89   | pair12,13 | stores 0,2,4,6
            #   ACT:   boot1 | pair67 | pair10,11 | pair14,15
            #          | routing bcast | stores 1,3,5
            #   SWDGE: pair23 | w_dn | x1 | x2 | ... | x7
            nc.sync.dma_start(out=boot_s[:, 0], in_=boot_d[0])
            nc.scalar.dma_start(out=boot_s[:, 1], in_=boot_d[1])
            nc.gpsimd.dma_start(out=wgu_s[:, 2:4], in_=wq_d[0, :, 2:4])
            nc.sync.dma_start(out=wgu_s[:, 4:6], in_=wq_d[1, :, 0:2])
            nc.scalar.dma_start(out=wgu_s[:, 6:8], in_=wq_d[1, :, 2:4])
            nc.sync.dma_start(out=wgu_s[:, 8:10], in_=wq_d[2, :, 0:2])
            nc.scalar.dma_start(out=wgu_s[:, 10:12], in_=wq_d[2, :, 2:4])
            nc.sync.dma_start(out=wgu_s[:, 12:14], in_=wq_d[3, :, 0:2])
            nc.scalar.dma_start(out=wgu_s[:, 14:16], in_=wq_d[3, :, 2:4])
            nc.gpsimd.dma_start(out=wdn_s, in_=wdn_d)
            for c in range(1, NT):
                nc.gpsimd.dma_start(out=x_s[:, c - 1], in_=x_d[c - 1])
            nc.scalar.dma_start(out=r_all, in_=rw_d.to_broadcast([P, T]))
            nc.gpsimd.dma_start(
                out=r_all[:, 2 * TC:],
                in_=rw_d[:, 2 * TC:].to_broadcast([P, T - 2 * TC]),
            )

            for tci in range(NT):
                t0 = tci * TC

                def mov(k, tci=tci):
                    if tci == 0:
                        return x0_mov(k)
                    return x_s[:, tci - 1, k, :]

                def stat(s, k):
                    if s < 2:
                        # slabs 0/1 live in the boot blob, all chunks
                        return boot_slab(s, k)
                    return wgu_s[:, s, k, :]

                r_c = r_all[:, t0:t0 + TC]
                hid = hid_pool.tile([P, KQ, TC], bf16)
                for qi in range(KQ):
                    gate_ps = psum_gu.tile([P, TC], f32, tag="gate")
                    up_ps = psum_gu.tile([P, TC], f32, tag="up")
                    for k in range(KH):
                        nc.tensor.matmul(
                            gate_ps,
                            stat(2 * qi, k),
                            mov(k),
                            start=(k == 0),
                            stop=(k == KH - 1),
                        )
                    for k in range(KH):
                        nc.tensor.matmul(
                            up_ps,
                            stat(2 * qi + 1, k),
                            mov(k),
                            start=(k == 0),
                            stop=(k == KH - 1),
                        )
                    tmp = tmp_pool.tile([P, TC], f32)
                    nc.scalar.activation(
                        tmp, gate_ps, mybir.ActivationFunctionType.Silu
                    )
                    nc.vector.tensor_mul(hid[:, qi, :], tmp, up_ps)

                ost = ost_pool.tile([P, NH, TC], bf16, tag="ost")
                for hi in range(NH):
                    if tci == NT - 1 and hi == NH - 1:
                        # final tile in two 256-token halves (separate PSUM
                        # tiles so the halves don't serialize): the first
                        # half streams out on the idle SWDGE queue while the
                        # PE finishes the second, so the end-of-kernel drain
                        # is one 64KB store instead of 128KB
                        for half in range(2):
                            o_ps = psum_o.tile([P, TC], f32)
                            hps = o_ps[:, 0:TC // 2]
                            hs = slice(half * (TC // 2), (half + 1) * (TC // 2))
                            for qi in range(KQ):
                                nc.tensor.matmul(
                                    hps,
                                    wdn_s[:, qi, hi * P:(hi + 1) * P],
                                    hid[:, qi, hs],
                                    start=(qi == 0),
                                    stop=(qi == KQ - 1),
                                )
                            nc.vector.tensor_mul(
                                ost[:, hi, hs], hps, r_c[:, hs]
                            )
                            eng = nc.gpsimd if half == 0 else nc.sync
                            eng.dma_start(
                                out=out_d[tci, :, hi:hi + 1, hs],
                                in_=ost[:, hi:hi + 1, hs],
                            )
                        continue
                    o_ps = psum_o.tile([P, TC], f32)
                    for qi in range(KQ):
                        nc.tensor.matmul(
                            o_ps,
                            wdn_s[:, qi, hi * P:(hi + 1) * P],
                            hid[:, qi, :],
                            start=(qi == 0),
                            stop=(qi == KQ - 1),
                        )
                    nc.vector.tensor_mul(ost[:, hi, :], o_ps, r_c)
                    if tci < NT - 1:
                        # one fat 1MB store per chunk (8KB rows), rings
                        # alternating so neither queue backs up
                        if hi == NH - 1:
                            eng = nc.sync if tci % 2 == 0 else nc.scalar
                            eng.dma_start(out=out_d[tci], in_=ost)
                    else:
                        # last chunk: split so the final piece after the last
                        # matmul is small and the two rings drain in parallel
                        if hi == 3:
                            nc.sync.dma_start(
                                out=out_d[tci, :, 0:4], in_=ost[:, 0:4]
                            )
                        elif hi == 6:
                            nc.scalar.dma_start(
                                out=out_d[tci, :, 4:7], in_=ost[:, 4:7]
                            )
    _split_waits(nc)
    return nc


def _get_nc():
    global _CACHED
    if _CACHED is None:
        _CACHED = _build()
    return _CACHED


def _pack_wgu(w):
    """(H, 2Q) -> (4, P, 4, KH, P) bf16, quad-major in first-use slab order
    (gate qi / up qi interleaved), so a 4-slab quad reads 8KB contiguous per
    partition."""
    import ml_dtypes

    w = np.asarray(w, dtype=np.float32)
    # (KH, P, n_blk, P): k-tile, partition, column block, column
    w4 = w.reshape(KH, P, NS, P)
    order = [b for qi in range(KQ) for b in (qi, KQ + qi)]
    slabs = w4.transpose(2, 1, 0, 3)[order]          # (NS, P, KH, P)
    quads = slabs.reshape(NS // 4, 4, P, KH, P).transpose(0, 2, 1, 3, 4)
    return np.ascontiguousarray(quads.astype(ml_dtypes.bfloat16))


def _make_in_maps(x, routing_weights, gate_up_proj, down_proj):
    import ml_dtypes

    x = np.asarray(x, dtype=np.float32)
    # x[t, h] -> xP[chunk, p, k, t_in] with h = k*P + p: 8KB rows per chunk
    xP = x.reshape(NT, TC, KH, P).transpose(0, 3, 2, 1).astype(ml_dtypes.bfloat16)
    rw = np.asarray(routing_weights, dtype=np.float32)
    in_maps = []
    for e in range(E):
        dn = np.asarray(down_proj[e], dtype=np.float32)
        wq = _pack_wgu(gate_up_proj[e])
        # boot half s: [x0.k(4s..4s+3) flat | slab s], 6KB per partition
        boot = np.concatenate(
            [
                xP[0].reshape(P, 2, KH // 2 * TC).transpose(1, 0, 2),
                wq[0, :, 0:2].reshape(P, 2, KH * P).transpose(1, 0, 2),
            ],
            axis=2,
        )
        in_maps.append({
            "xT": np.ascontiguousarray(xP[1:]),
            "w_gu": wq,
            "boot": np.ascontiguousarray(boot),
            # w_dn[p, qi, h] = down_proj[qi*P + p, h]: 16KB rows
            "w_dn": np.ascontiguousarray(
                dn.reshape(KQ, P, H).transpose(1, 0, 2).astype(ml_dtypes.bfloat16)
            ),
            "rw": np.ascontiguousarray(rw[:, e].reshape(1, T)),
        })
    return in_maps


def _reduce_out(res):
    total = np.zeros((NT, P, NH, TC), dtype=np.float32)
    for r in res.results:
        total += r["out"].astype(np.float32).reshape(NT, P, NH, TC)
    # [chunk, p, hi, t_in] -> (T, H) with h = hi*P + p
    return np.ascontiguousarray(
        total.transpose(0, 3, 2, 1).reshape(T, H)
    )


def kernel(x, routing_weights, gate_up_proj, down_proj):
    from concourse.bass_utils import run_bass_kernel_spmd

    nc = _get_nc()
    in_maps = _make_in_maps(x, routing_weights, gate_up_proj, down_proj)
    res = run_bass_kernel_spmd(nc, in_maps, core_ids=list(range(E)))
    return _reduce_out(res)


# revision 35
# speedup vs baseline: 1.0140x; 1.0048x over previous
"""MoE experts kernel for TRN2, expert-parallel over 8 NeuronCores.

Reference computation (T=4096, E=8, H=1024, Q=1024):
    gate_up = einsum('th,ehq->teq', x, gate_up_proj)      # (T, E, 2Q)
    gate, up = split(gate_up, 2, axis=-1)
    hidden = silu(gate) * up                              # (T, E, Q)
    expert_outputs = einsum('teq,eqh->teh', hidden, down_proj)
    out = einsum('teh,te->th', expert_outputs, routing_weights)

Sharding: expert-parallel. Core e computes its expert's full contribution
r[:, e] * (silu(x @ Wgu_gate) * (x @ Wgu_up)) @ Wdn  for all T tokens,
entirely in feature-major layout (features on partitions, tokens on the
free axis) so no on-device transposes are needed; the host sums the 8
partial outputs (the expert-parallel all-reduce) and transposes back.

Per-core cost model (measured):
  - 1536 bf16 matmuls of [128 contraction x 512 moving] at ~216ns each
    = 332us PE busy; 216ns is the effective clock floor (same-stationary
    matmuls are no faster). fp8 is no help twice over: its quantization
    error (3.8-6.5% end-to-end) eats the 2e-2 gate's margin, AND
    DoubleRow measures at most ~1.44x, so one residual-compensation
    matmul already makes it a net loss vs bf16.
  - Each dma_start costs ~2us fixed (completion receipt) + bytes at an
    effective rate set by the DRAM-side descriptor row size (~136GB/s at
    2KB rows), serialized per queue. So: few fat DMAs, with DRAM layouts
    host-packed so every transfer reads 4-16KB contiguous per partition.
    Three queues run concurrently: SP (nc.sync), Activation (nc.scalar),
    and SWDGE (nc.gpsimd).
  - The PE clock ramps over ~8 matmuls (~427ns each) after idling;
    dummy matmuls during the unavoidable initial DMA wait pay that cost
    off the critical path. A mid-stream starve >1us re-ramps.
  - ~7.5us fixed framework prologue before any DMA issues; ~1.8us
    epilogue barrier after the last store's semaphore.
"""

import sys

for _p in ("/opt/trn_rl_repo", "/root/.axon_site/_ro/trn_rl_repo"):
    if _p not in sys.path:
        sys.path.insert(0, _p)

import numpy as np

T, E, H, Q = 4096, 8, 1024, 1024
P = 128          # partitions
TC = 512         # token chunk (= one PSUM bank of fp32)
NT = T // TC     # 8 token chunks
KH = H // P      # 8 contraction tiles for the gate_up matmul
KQ = Q // P      # 8 contraction tiles for the down matmul
NH = H // P      # 8 output-feature tiles
NS = 2 * Q // P  # 16 gate_up weight slabs (gate qi / up qi interleaved)
N_WARM = 18      # PE clock-warmup dummies covering the first-DMA wait
BOOT = KH * TC // 2 + KH * P  # 3072: half of x chunk 0 + one slab, per queue

_CACHED = None


def _split_waits(nc, max_waits=1):
    """Walrus codegen for several TRN2 ISA structs accepts only one sync-wait
    per instruction ("Too many sync wait commands"). Splitting is safe: a
    same-engine NoOp earlier in the (FIFO) stream carrying the extra waits
    blocks the stream at the same point the original multi-wait would have."""
    import concourse.mybir as mybir

    for f in nc.m.functions:
        for blk in f.blocks:
            newlist, changed = [], False
            for inst in blk.instructions:
                si = inst.sync_info
                if si is not None and si.on_wait and len(si.on_wait) > max_waits:
                    extra = si.on_wait[:-max_waits]
                    keep = si.on_wait[-max_waits:]
                    inst.sync_info = mybir.SyncInfo(
                        on_wait=list(keep), on_update=list(si.on_update or [])
                    )
                    for j, w in enumerate(extra):
                        nop = mybir.InstNoOp(
                            name=f"{inst.name}-wn{j}", engine=inst.engine
                        )
                        nop.sync_info = mybir.SyncInfo(on_wait=[w], on_update=[])
                        newlist.append(nop)
                    changed = True
                newlist.append(inst)
            if changed:
                blk.instructions = newlist


def _build():
    import concourse.bass as bass
    import concourse.mybir as mybir
    import concourse.tile as tile

    nc = bass.Bass("TRN2", target_bir_lowering=False, debug=False, num_devices=E)

    f32 = mybir.dt.float32
    # bf16: same PE rate as fp32r (1 cycle/row for moving >= 256) but half
    # the HBM traffic and half-width weight loads; quantization adds ~0.3%
    # relative error, well inside the 2e-2 gate.
    bf16 = mybir.dt.bfloat16

    # All DRAM layouts are host-packed so each DMA reads big contiguous
    # per-partition rows (see _make_in_maps):
    #   w_gu: quad-major [quad, P, slab-in-quad, KH, P]  -> 8KB rows/quad
    #   w_dn: [P, KQ, H]                                 -> 16KB rows
    #   xT:   [chunk, P, KH, TC]                         -> 8KB rows/chunk
    #   out:  [chunk, P, NH, TC]                         -> 8KB rows/chunk
    wq_d = nc.dram_tensor(
        "w_gu", [NS // 4, P, 4, KH, P], bf16, kind="ExternalInput"
    ).ap()
    wdn_d = nc.dram_tensor("w_dn", [P, KQ, H], bf16, kind="ExternalInput").ap()
    # chunks 1..7 of x; chunk 0 rides in the boot blob
    x_d = nc.dram_tensor(
        "xT", [NT - 1, P, KH, TC], bf16, kind="ExternalInput"
    ).ap()
    # boot blob: everything the first k-sweep needs, packed per partition as
    # [x0.k0-3 | slab0] and [x0.k4-7 | slab1] so ONE fat DMA per HWDGE queue
    # (6KB rows) pays the ~2us fixed cost once and lands it all together
    boot_d = nc.dram_tensor("boot", [2, P, BOOT], bf16, kind="ExternalInput").ap()
    rw_d = nc.dram_tensor("rw", [1, T], mybir.dt.float32, kind="ExternalInput").ap()
    # bf16 output: the host upcasts and sums the 8 expert partials in fp32;
    # the extra ~0.2% quantization is inside the 2e-2 budget and halves the
    # store traffic (shrinks the end-of-kernel DMA drain).
    out_d = nc.dram_tensor("out", [NT, P, NH, TC], bf16, kind="ExternalOutput").ap()

    from contextlib import ExitStack

    with tile.TileContext(nc) as tc:
        with ExitStack() as es:
            consts = es.enter_context(tc.tile_pool(name="consts", bufs=1))
            psum_gu = es.enter_context(tc.tile_pool(name="psum_gu", bufs=2, space="PSUM"))
            psum_o = es.enter_context(tc.tile_pool(name="psum_o", bufs=4, space="PSUM"))
            hid_pool = es.enter_context(tc.tile_pool(name="hid", bufs=2))
            tmp_pool = es.enter_context(tc.tile_pool(name="tmp", bufs=2))
            ost_pool = es.enter_context(tc.tile_pool(name="ost", bufs=2))
            wgu_s = consts.tile([P, NS, KH, P], bf16)
            wdn_s = consts.tile([P, KQ, H], bf16)
            # all x chunks stay SBUF-resident (62KB/partition with the boot
            # blob): no mid-stream x traffic at all after the startup loads
            x_s = consts.tile([P, NT - 1, KH, TC], bf16)
            boot_s = consts.tile([P, 2, BOOT], bf16)
            r_all = consts.tile([P, T], f32)

            def x0_mov(k):
                half, kk = divmod(k, 4)
                return boot_s[:, half, kk * TC:(kk + 1) * TC]

            def boot_slab(s, k):
                return boot_s[:, s, 4 * TC + k * P:4 * TC + (k + 1) * P]

            # PE p-state warmup: the engine idles from the end of the
            # framework prologue until the first weights+x land (~12.5us);
            # matmuls on a zeroed scratch tile during that window ramp the
            # clock so the real stream starts at full speed. Results land
            # in a PSUM bank that every real accumulation group resets
            # with start=True.
            dmy = consts.tile([P, 4, P], bf16)
            nc.gpsimd.memset(dmy, 0)
            warm_ps = psum_gu.tile([P, TC], f32, tag="gate")
            for _ in range(N_WARM):
                nc.tensor.matmul(
                    warm_ps, dmy[:, 0, :], dmy.rearrange("p a b -> p (a b)"),
                    start=True, stop=True,
                )

            # Startup schedule. The HBM aggregate (~320GB/s) is the binding
            # constraint and the queues round-robin for it, so: one boot-blob
            # DMA per HWDGE queue delivers the whole first k-sweep at once;
            # the rest of the weight stream (consumed at ~148GB/s by the PE)
            # is split 50/50 over the two fast queues as slab pairs in
            # consumption order; the slower SWDGE queue carries only traffic
            # whose deadline is far out, with x chunks 2-7 queued BEHIND
            # w_dn so they cannot steal bandwidth during the weight window.
            #   SP:    boot0 | pair45 | pair89   | pair12,13 | stores 0,2,4,6
            #   ACT:   boot1 | pair67 | pair10,11 | pair14,15
            #          | routing bcast | stores 1,3,5
            #   SWDGE: pair23 | w_dn | x1 | x2 | ... | x7
            nc.sync.dma_start(out=boot_s[:, 0], in_=boot_d[0])
            nc.scalar.dma_start(out=boot_s[:, 1], in_=boot_d[1])
            nc.gpsimd.dma_start(out=wgu_s[:, 2:4], in_=wq_d[0, :, 2:4])
            nc.sync.dma_start(out=wgu_s[:, 4:6], in_=wq_d[1, :, 0:2])
            nc.scalar.dma_start(out=wgu_s[:, 6:8], in_=wq_d[1, :, 2:4])
            nc.sync.dma_start(out=wgu_s[:, 8:10], in_=wq_d[2, :, 0:2])
            nc.scalar.dma_start(out=wgu_s[:, 10:12], in_=wq_d[2, :, 2:4])
            nc.sync.dma_start(out=wgu_s[:, 12:14], in_=wq_d[3, :, 0:2])
            nc.scalar.dma_start(out=wgu_s[:, 14:16], in_=wq_d[3, :, 2:4])
            nc.gpsimd.dma_start(out=wdn_s, in_=wdn_d)
            for c in range(1, NT):
                nc.gpsimd.dma_start(out=x_s[:, c - 1], in_=x_d[c - 1])
            nc.scalar.dma_start(out=r_all, in_=rw_d.to_broadcast([P, T]))
            nc.gpsimd.dma_start(
                out=r_all[:, 2 * TC:],
                in_=rw_d[:, 2 * TC:].to_broadcast([P, T - 2 * TC]),
            )

            for tci in range(NT):
                t0 = tci * TC

                def mov(k, tci=tci):
                    if tci == 0:
                        return x0_mov(k)
                    return x_s[:, tci - 1, k, :]

                def stat(s, k):
                    if s < 2:
                        # slabs 0/1 live in the boot blob, all chunks
                        return boot_slab(s, k)
                    return wgu_s[:, s, k, :]

                r_c = r_all[:, t0:t0 + TC]
                hid = hid_pool.tile([P, KQ, TC], bf16)
                for qi in range(KQ):
                    gate_ps = psum_gu.tile([P, TC], f32, tag="gate")
                    up_ps = psum_gu.tile([P, TC], f32, tag="up")
                    for k in range(KH):
                        nc.tensor.matmul(
                            gate_ps,
                            stat(2 * qi, k),
                            mov(k),
                            start=(k == 0),
                            stop=(k == KH - 1),
                        )
                    for k in range(KH):
                        nc.tensor.matmul(
                            up_ps,
                            stat(2 * qi + 1, k),
                            mov(k),
                            start=(k == 0),
                            stop=(k == KH - 1),
                        )
                    tmp = tmp_pool.tile([P, TC], f32)
                    nc.scalar.activation(
                        tmp, gate_ps, mybir.ActivationFunctionType.Silu
                    )
                    nc.vector.tensor_mul(hid[:, qi, :], tmp, up_ps)

                ost = ost_pool.tile([P, NH, TC], bf16, tag="ost")
                for hi in range(NH):
                    o_ps = psum_o.tile([P, TC], f32)
                    for qi in range(KQ):
                        nc.tensor.matmul(
                            o_ps,
                            wdn_s[:, qi, hi * P:(hi + 1) * P],
                            hid[:, qi, :],
                            start=(qi == 0),
                            stop=(qi == KQ - 1),
                        )
                    nc.vector.tensor_mul(ost[:, hi, :], o_ps, r_c)
                    if tci < NT - 1:
                        # one fat 1MB store per chunk (8KB rows), rings
                        # alternating so neither queue backs up
                        if hi == NH - 1:
                            eng = nc.sync if tci % 2 == 0 else nc.scalar
                            eng.dma_start(out=out_d[tci], in_=ost)
                    else:
                        # last chunk: split so the final piece after the last
                        # matmul is small and the two rings drain in parallel
                        if hi == 3:
                            nc.sync.dma_start(
                                out=out_d[tci, :, 0:4], in_=ost[:, 0:4]
                            )
                        elif hi == 6:
                            nc.scalar.dma_start(
                                out=out_d[tci, :, 4:7], in_=ost[:, 4:7]
                            )
                        elif hi == 7:
                            nc.sync.dma_start(
                                out=out_d[tci, :, 7:8], in_=ost[:, 7:8]
                            )
    _split_waits(nc)
    return nc


def _get_nc():
    global _CACHED
    if _CACHED is None:
        _CACHED = _build()
    return _CACHED


def _pack_wgu(w):
    """(H, 2Q) -> (4, P, 4, KH, P) bf16, quad-major in first-use slab order
    (gate qi / up qi interleaved), so a 4-slab quad reads 8KB contiguous per
    partition."""
    import ml_dtypes

    w = np.asarray(w, dtype=np.float32)
    # (KH, P, n_blk, P): k-tile, partition, column block, column
    w4 = w.reshape(KH, P, NS, P)
    order = [b for qi in range(KQ) for b in (qi, KQ + qi)]
    slabs = w4.transpose(2, 1, 0, 3)[order]          # (NS, P, KH, P)
    quads = slabs.reshape(NS // 4, 4, P, KH, P).transpose(0, 2, 1, 3, 4)
    return np.ascontiguousarray(quads.astype(ml_dtypes.bfloat16))


def _make_in_maps(x, routing_weights, gate_up_proj, down_proj):
    import ml_dtypes

    x = np.asarray(x, dtype=np.float32)
    # x[t, h] -> xP[chunk, p, k, t_in] with h = k*P + p: 8KB rows per chunk
    xP = x.reshape(NT, TC, KH, P).transpose(0, 3, 2, 1).astype(ml_dtypes.bfloat16)
    rw = np.asarray(routing_weights, dtype=np.float32)
    in_maps = []
    for e in range(E):
        dn = np.asarray(down_proj[e], dtype=np.float32)
        wq = _pack_wgu(gate_up_proj[e])
        # boot half s: [x0.k(4s..4s+3) flat | slab s], 6KB per partition
        boot = np.concatenate(
            [
                xP[0].reshape(P, 2, KH // 2 * TC).transpose(1, 0, 2),
                wq[0, :, 0:2].reshape(P, 2, KH * P).transpose(1, 0, 2),
            ],
            axis=2,
        )
        in_maps.append({
            "xT": np.ascontiguousarray(xP[1:]),
            "w_gu": wq,
            "boot": np.ascontiguousarray(boot),
            # w_dn[p, qi, h] = down_proj[qi*P + p, h]: 16KB rows
            "w_dn": np.ascontiguousarray(
                dn.reshape(KQ, P, H).transpose(1, 0, 2).astype(ml_dtypes.bfloat16)
            ),
            "rw": np.ascontiguousarray(rw[:, e].reshape(1, T)),
        })
    return in_maps


def _reduce_out(res):
    total = np.zeros((NT, P, NH, TC), dtype=np.float32)
    for r in res.results:
        total += r["out"].astype(np.float32).reshape(NT, P, NH, TC)
    # [chunk, p, hi, t_in] -> (T, H) with h = hi*P + p
    return np.ascontiguousarray(
        total.transpose(0, 3, 2, 1).reshape(T, H)
    )


def kernel(x, routing_weights, gate_up_proj, down_proj):
    from concourse.bass_utils import run_bass_kernel_spmd

    nc = _get_nc()
    in_maps = _make_in_maps(x, routing_weights, gate_up_proj, down_proj)
    res = run_bass_kernel_spmd(nc, in_maps, core_ids=list(range(E)))
    return _reduce_out(res)
